# revision 22
# baseline (speedup 1.0000x reference)
"""TRN2 Bass kernel for nn_BetweenClusterFC.

Computes out[n] = sum_f (emb_1 @ W1 + b1)[n,f] * (emb_2 @ W2 + b2)[n,f]
for emb_1/emb_2 [32768, 1024] fp32, W [1024, 512], b [512], out [32768] fp32.

Sharding: data-parallel over the 8 NeuronCores — each core handles 4096 rows;
W1/W2 replicated. No cross-core communication; outputs concatenated on the
host.

Strategy (v3 — single-pass fp16, DMA-descriptor-lean):
  - The rel-err budget for this problem is 2e-2; single-pass fp16 matmuls
    land at ~3.4e-4 (measured on HW), so one fp16 matmul per (tile, kc,
    input) = 512 MMs of N=512 per core runs at the warm-PE stream roofline
    (216 ns/MM measured).
  - Embeddings are relaid out host-side to [group, p, kc, r] so each
    2-row-tile group DMA is 128 descriptors of 4KB contiguous per partition
    (vs 1024x512B from a plain [D, N] transpose — those took 1.6us of sync-
    engine issue time each and stalled the PE stream). Weights are [p, kc, f]
    -> one 128-descriptor DMA per weight matrix.
  - Per 128-row tile: 16 interleaved matmuls accumulate h1/h2 into two PSUM
    banks; the scalar engine stages h1 PSUM->SBUF (DVE may read only one
    PSUM operand); one fused DVE scalar_tensor_tensor computes
    prod = h1*h2 and accum_out = sum_f into acc[:, rt]. Biases are zero in
    this problem; a general bias variant is compiled only if b1/b2 != 0.
  - acc [128 rows-in-tile, 32 tiles] is DMA'd out raw and transposed on the
    host (free), eliminating the PE-transpose + copy + strided-store tail.
  - fp16 warmup matmuls on a zeroed tile bridge the startup-DMA window so
    real matmuls start at the un-throttled PE clock (HAM un-throttles after
    ~3.4us of sustained PE activity).
"""

import sys
import time

import numpy as np

if "/opt/trn_rl_repo" not in sys.path:
    sys.path.insert(0, "/opt/trn_rl_repo")

import concourse.mybir as mybir
import concourse.tile as tile
from concourse import bacc
from concourse.bass_utils import run_bass_kernel_spmd

F32 = mybir.dt.float32
F16 = mybir.dt.float16

N = 32768
D = 1024
F = 512
P = 128
NCORES = 8
R = N // NCORES   # rows per core
RT = R // P       # 128-row tiles per core
KC = D // P       # contraction chunks
NFIRST = 2        # single-tile e-DMA groups at the head (fast first arrival)
GRP = 2           # row-tiles per e-DMA group for the rest
NG = (RT - NFIRST) // GRP  # rest e-DMA groups
NWARM = 6         # fp16 warmup matmuls bridging the startup DMA window
FUSE_STT = False  # fused DVE mult+reduce crashed HW intermittently; keep off

_CACHE = {}


def _build_program(with_bias=False, rows=R):
    rt_count = rows // P
    ng = (rt_count - NFIRST) // GRP
    nc = bacc.Bacc("TRN2", target_bir_lowering=False, debug=False)

    def din(name, shape, dt=F16):
        return nc.dram_tensor(name, shape, dt, kind="ExternalInput").ap()

    # host-prearranged layouts: e [group, p, kc, r-in-group], w [p, kc, f]
    e1f = din("e1f", [NFIRST, P, KC, P])
    e2f = din("e2f", [NFIRST, P, KC, P])
    e1h = din("e1h", [ng, P, KC, GRP * P])
    e2h = din("e2h", [ng, P, KC, GRP * P])
    # W1/W2 interleaved per-kc so one DMA delivers both inputs' chunk k
    wb = din("wb", [P, KC, 2, F])
    if with_bias:
        b1 = din("b1", [F], F32)
        b2 = din("b2", [F], F32)
    # out[p, rt] = result for row rt*128+p; transposed host-side
    out = nc.dram_tensor("out", [P, rt_count], F32, kind="ExternalOutput").ap()

    mult = mybir.AluOpType.mult
    add = mybir.AluOpType.add

    with tile.TileContext(nc) as tc:
        with (
            tc.tile_pool(name="consts", bufs=1) as consts,
            tc.tile_pool(name="epool", bufs=1) as epool,
            tc.tile_pool(name="hpool", bufs=2) as hpool,
            tc.tile_pool(name="fin", bufs=1) as fin_pool,
            tc.tile_pool(name="w_psum", bufs=1, space="PSUM") as w_psum,
            tc.tile_pool(name="h_psum", bufs=3, space="PSUM") as h_psum,
        ):
            wsb = consts.tile([P, KC, 2, F], F16, tag="wsb")
            ef = [
                [epool.tile([P, KC, P], F16, tag=f"e{j}f{t}", name=f"e{j}f{t}")
                 for t in range(NFIRST)]
                for j in range(2)
            ]
            eg = [
                [epool.tile([P, KC, GRP * P], F16, tag=f"e{j}g{g}",
                            name=f"e{j}g{g}")
                 for g in range(ng)]
                for j in range(2)
            ]

            # Startup is HBM-paced: tile 0 needs ~2.5MB (both W + first e
            # tiles). Per-DMA fixed latency (~1.5us, ~0.3us pipelined) makes
            # small chunks slow, so: weights in two 1MB halves — the first
            # half on the gpsimd SWDGE ring (issues earliest, transfers in
            # parallel with the sync ring), the second on the sync ring after
            # the first e tiles. Each DMA is 128 descriptors of >=2KB
            # contiguous per partition.
            nc.gpsimd.dma_start(wsb[:, :KC // 2], wb[:, :KC // 2])
            nc.sync.dma_start(ef[0][0][:], e1f[0])
            nc.sync.dma_start(ef[1][0][:], e2f[0])
            nc.sync.dma_start(wsb[:, KC // 2:], wb[:, KC // 2:])
            for t in range(1, NFIRST):
                nc.sync.dma_start(ef[0][t][:], e1f[t])
                nc.sync.dma_start(ef[1][t][:], e2f[t])
            for g in range(ng):
                nc.sync.dma_start(eg[0][g][:], e1h[g])
                nc.sync.dma_start(eg[1][g][:], e2h[g])

            if with_bias:
                b1_bc = consts.tile([P, F], F32, tag="b1")
                nc.gpsimd.dma_start(b1_bc[:], b1[None, :].to_broadcast((P, F)))
                b2_bc = consts.tile([P, F], F32, tag="b2")
                nc.gpsimd.dma_start(b2_bc[:], b2[None, :].to_broadcast((P, F)))

            # fp16 warmup matmuls bridge the startup-DMA window so the first
            # real matmuls run at the un-throttled PE clock; alternate two
            # PSUM banks so consecutive warmups overlap fill/drain
            warm16 = consts.tile([P, F], F16, tag="warm16")
            nc.vector.memset(warm16[:], 0.0)
            warm_ps = [w_psum.tile([P, F], F32, tag=f"warm{i}", name=f"warm{i}")
                       for i in range(2)]
            for i in range(NWARM):
                nc.tensor.matmul(
                    warm_ps[i % 2][:], lhsT=warm16[:, :P], rhs=warm16[:],
                    start=True, stop=True,
                )

            acc = fin_pool.tile([P, rt_count], F32, tag="acc")

            for rt in range(rt_count):
                if rt < NFIRST:
                    lhs_of = lambda j, kc, t=rt: ef[j][t][:, kc, :]
                else:
                    g, ri = divmod(rt - NFIRST, GRP)
                    lhs_of = lambda j, kc, g=g, ri=ri: \
                        eg[j][g][:, kc, ri * P:(ri + 1) * P]
                hps = [
                    h_psum.tile([P, F], F32, tag=f"h{j}", name=f"hp{j}_{rt}")
                    for j in range(2)
                ]
                for kc in range(KC):
                    for j in range(2):
                        nc.tensor.matmul(
                            hps[j][:],
                            lhsT=lhs_of(j, kc),
                            rhs=wsb[:, kc, j, :],
                            start=(kc == 0),
                            stop=(kc == KC - 1),
                        )

                if with_bias:
                    hts = []
                    for j, b_bc in enumerate((b1_bc, b2_bc)):
                        ht = hpool.tile([P, F], F32, tag=f"ht{j}")
                        nc.vector.tensor_tensor(ht[:], hps[j][:], b_bc[:], add)
                        hts.append(ht)
                    in0, in1 = hts[0][:], hts[1][:]
                else:
                    # DVE can read at most one PSUM operand; stage h0 in SBUF
                    # via the scalar engine (close to PSUM, off the DVE path)
                    h0sb = hpool.tile([P, F], F32, tag="h0sb")
                    nc.scalar.activation(
                        h0sb[:], hps[0][:], mybir.ActivationFunctionType.Copy)
                    in0, in1 = h0sb[:], hps[1][:]

                prod = hpool.tile([P, F], F32, tag="prod")
                if FUSE_STT:
                    nc.vector.scalar_tensor_tensor(
                        prod[:], in0, 1.0, in1, op0=mult, op1=mult,
                        accum_out=acc[:, rt:rt + 1],
                    )
                else:
                    nc.vector.tensor_tensor(prod[:], in0, in1, mult)
                    nc.vector.tensor_reduce(
                        acc[:, rt:rt + 1], prod[:],
                        axis=mybir.AxisListType.X, op=add,
                    )

            nc.sync.dma_start(out, acc[:])

    nc.compile()
    return nc


def _get_program(with_bias=False):
    key = ("bias" if with_bias else "fast")
    if key not in _CACHE:
        _CACHE[key] = _build_program(with_bias=with_bias)
    return _CACHE[key]


def _prep_e(emb):
    # [N, D] fp32 -> per-core ([NFIRST, p, kc, 128], [ng, p, kc, GRP*128])
    # fp16; contiguous per (group, partition) for 128-descriptor DMAs
    et = np.ascontiguousarray(
        np.asarray(emb, dtype=np.float32).T).astype(np.float16)
    # et [D, N]: [kc*128+p, c*R + rt*P + r]
    v = et.reshape(KC, P, NCORES, RT, P).transpose(2, 3, 1, 0, 4)
    # v [c, rt, p, kc, r]
    first = np.ascontiguousarray(v[:, :NFIRST])
    rest = np.ascontiguousarray(
        v[:, NFIRST:].reshape(NCORES, NG, GRP, P, KC, P)
        .transpose(0, 1, 3, 4, 2, 5)
        .reshape(NCORES, NG, P, KC, GRP * P))
    return first, rest


def _prep_w(W1, W2):
    # -> [p, kc, j, f] fp16: one 2KB-contiguous chunk per (partition, kc)
    w = np.stack([
        np.asarray(W1, dtype=np.float32).astype(np.float16),
        np.asarray(W2, dtype=np.float32).astype(np.float16),
    ])  # [j, kc*128+p, f]
    return np.ascontiguousarray(
        w.reshape(2, KC, P, F).transpose(2, 1, 0, 3))


def make_in_maps(emb_1, emb_2, W1, b1, W2, b2, with_bias=False):
    e1f, e1r = _prep_e(emb_1)
    e2f, e2r = _prep_e(emb_2)
    wb = _prep_w(W1, W2)
    maps = []
    for c in range(NCORES):
        m = {"e1f": e1f[c], "e2f": e2f[c], "e1h": e1r[c], "e2h": e2r[c],
             "wb": wb}
        if with_bias:
            m["b1"] = np.ascontiguousarray(np.asarray(b1, dtype=np.float32))
            m["b2"] = np.ascontiguousarray(np.asarray(b2, dtype=np.float32))
        maps.append(m)
    return maps


def kernel(emb_1, emb_2, W1, b1, W2, b2, **_unused):
    with_bias = bool(np.any(np.asarray(b1)) or np.any(np.asarray(b2)))
    nc = _get_program(with_bias)
    in_maps = make_in_maps(emb_1, emb_2, W1, b1, W2, b2, with_bias=with_bias)
    last_err = None
    for attempt in range(3):
        try:
            res = run_bass_kernel_spmd(nc, in_maps, list(range(NCORES))).results
            # out[p, rt] -> rows rt*128+p
            return np.concatenate(
                [res[c]["out"].T.reshape(R) for c in range(NCORES)])
        except Exception as e:  # transient NRT/axon failures observed; retry
            last_err = e
            time.sleep(2.0 * (attempt + 1))
    raise last_err
